# revision 25
# baseline (speedup 1.0000x reference)
"""Trainium2 (8 NeuronCores, SPMD) kernel for windowed multi-head attention
with relative position bias (Swin-3D style block).

Strategy: pure data-parallel over the B=32 window axis — 4 windows per core,
no collectives. Per core:
  phase 1: qkv projection.  q,k produced TRANSPOSED (feature-on-partition)
           for the score matmuls; v produced natural (token-on-partition).
  phase 2: per (head-pair, window): scores^T via ROW-TILED 64x128 matmul
           pairs (two heads run concurrently on the two 64-row halves of the
           PE array), exp on ScalarE, multiply by exp(bias)^T (resident in
           SBUF), PV via COL-TILED 128x64 matmul pairs (two heads' v share
           the array column halves, writing one PSUM bank; evacuated
           unnormalized by ScalarE).  Softmax denominators come from a
           4-way col-tiled (128x32 mode) ones-matmul covering 4 heads at
           once; 1/S is broadcast across partitions via a DRAM round-trip
           on otherwise-idle DMA engines (or, for the final group, via K=1
           outer-product matmuls on the then-idle PE to dodge the DMA
           semaphore latency) and applied to the unnormalized PV pair in a
           single [128,512] multiply.
  phase 3: output projection from attn-out^T tiles; DMAed out.
  PSUM: 4 banks score/sums ring + 3 banks qkv/proj fill ring + 1 PV.
  The PE is pre-warmed with dummy matmuls during the DMA prologue so the
  HAM clock gate reaches 2.4 GHz before the first real qkv group.

All matmul operands are bf16 (full TensorE rate); accumulation fp32 in PSUM.
The softmax scale is folded into the q weights on the host. exp(s+b) is
computed as exp(s)*exp(b).  The exp(bias) table (12 heads x [512,512], bf16)
stays RESIDENT in SBUF (loaded once, ~6.3MB) instead of being re-streamed
per window; to make room, qkT/v/attn-out buffers are double-buffered over a
2-window rotation instead of holding all 4 windows.
v/proj biases enter the output linearly and are applied on the host (they
are zeros for this problem's inputs anyway).
"""

import numpy as np
import ml_dtypes

B, NTOK, DIM = 32, 512, 768
NH, HD = 12, 64
NCORES = 8
BW = B // NCORES          # 4 windows per core
SCALE = HD ** -0.5
KT = NTOK // 128          # 4 token tiles
FT = DIM // 128           # 6 feature tiles
NHP = NH // 2             # 6 head pairs

TRACE = False             # set by test.py to capture neuron-profile timing
LAST_RESULT = None        # BassKernelResults of the last run (for profiling)

_nc_cache = {}


def _build(has_bqk: bool):
    import concourse.mybir as mybir
    import concourse.tile as tile
    from concourse import bacc
    from contextlib import ExitStack

    dt = mybir.dt
    bf16, f32 = dt.bfloat16, dt.float32
    AF = mybir.ActivationFunctionType

    nc = bacc.Bacc("TRN2", target_bir_lowering=False, debug=False)
    xT_d = nc.declare_dram_parameter("xT", [BW, DIM, NTOK], bf16, False)
    wq_d = nc.declare_dram_parameter("wqkvT", [DIM, 3 * DIM], bf16, False)
    wp_d = nc.declare_dram_parameter("wprojT", [DIM, DIM], bf16, False)
    eb_d = nc.declare_dram_parameter("expb", [128, NH, KT, NTOK], bf16, False)
    bq_d = nc.declare_dram_parameter("bqk", [128, 2 * FT], f32, False)
    out_d = nc.declare_dram_parameter("out", [BW, NTOK, DIM], f32, True)

    ctx = ExitStack()
    with ctx:
        tc = ctx.enter_context(tile.TileContext(nc))
        const = ctx.enter_context(tc.tile_pool(name="const", bufs=1))
        xpool = ctx.enter_context(tc.tile_pool(name="xT", bufs=2))
        empool = ctx.enter_context(tc.tile_pool(name="expm", bufs=6))
        uopool = ctx.enter_context(tc.tile_pool(name="unorm", bufs=3))
        rbpool = ctx.enter_context(tc.tile_pool(name="rb", bufs=2))
        sbcpool = ctx.enter_context(tc.tile_pool(name="sbc", bufs=2))
        rcpool = ctx.enter_context(tc.tile_pool(name="rc", bufs=2))
        opool = ctx.enter_context(tc.tile_pool(name="osb", bufs=2))
        ps_s = ctx.enter_context(tc.tile_pool(name="ps_s", bufs=4, space="PSUM"))
        ps_f = ctx.enter_context(tc.tile_pool(name="ps_f", bufs=3, space="PSUM"))
        ps_pv = ctx.enter_context(tc.tile_pool(name="ps_pv", bufs=1, space="PSUM"))
        ps_sum = ps_s   # sums tiles share the score pool's tag ring
        drampool = ctx.enter_context(tc.tile_pool(name="rdram", bufs=3, space="DRAM"))

        # ---- resident constants -------------------------------------------
        # Split the big weight loads so the first matmuls can start while the
        # rest of the inputs stream in (the prologue is HBM-bandwidth bound).
        wq_sb = const.tile([128, FT, 3 * DIM], bf16)
        wq_r = wq_d[:, :].rearrange("(k p) c -> p k c", p=128)
        wp_sb = const.tile([128, FT, DIM], bf16)
        eb_sb = const.tile([128, NH, KT, NTOK], bf16)   # exp(bias), all heads
        bqk_sb = const.tile([128, 2 * FT], f32)
        qkT = const.tile([128, 2, 2 * FT, NTOK], bf16)  # q,k transposed, 2-window rot
        vsb = const.tile([128, 2, KT, NH, HD], bf16)    # v natural, 2-window rot
        aoT = const.tile([128, 2, FT, NTOK], bf16)      # attn out^T, 2-window rot
        ones_sb = const.tile([128, 1], bf16)
        ones_f32 = const.tile([128, 64], f32)

        # first qk-group's weight block + first window's x first: they gate
        # the first matmul
        nc.sync.dma_start(out=wq_sb[:, :, 0:256], in_=wq_r[:, :, 0:256])
        nc.vector.memset(ones_sb, 1.0)
        nc.vector.memset(ones_f32, 1.0)
        nc.sync.dma_start(out=bqk_sb, in_=bq_d[:, :])
        xw0 = xpool.tile([128, FT, NTOK], bf16, name="xw", tag="xw")
        nc.sync.dma_start(
            out=xw0[:, 0:3, :],
            in_=xT_d[0, 0:384, :].rearrange("(k p) t -> p k t", p=128),
        )
        nc.sync.dma_start(
            out=xw0[:, 3:6, :],
            in_=xT_d[0, 384:768, :].rearrange("(k p) t -> p k t", p=128),
        )
        nc.sync.dma_start(out=wq_sb[:, :, 256:DIM], in_=wq_r[:, :, 256:DIM])
        nc.sync.dma_start(out=wq_sb[:, :, DIM:2 * DIM], in_=wq_r[:, :, DIM:2 * DIM])
        nc.sync.dma_start(out=wq_sb[:, :, 2 * DIM:], in_=wq_r[:, :, 2 * DIM:])
        # exp(bias) for head pair 0 first (gates first attention), rest after
        nc.sync.dma_start(out=eb_sb[:, 0:2], in_=eb_d[:, 0:2])
        nc.sync.dma_start(
            out=wp_sb, in_=wp_d[:, :].rearrange("(k p) c -> p k c", p=128)
        )
        for hp in range(1, NHP):
            nc.sync.dma_start(out=eb_sb[:, 2 * hp:2 * hp + 2], in_=eb_d[:, 2 * hp:2 * hp + 2])

        # pre-warm the PE while the prologue DMAs stream: the HAM clock
        # gate needs ~3.4us of activity before it unthrottles 1.2->2.4 GHz,
        # so burn it on dummy matmuls instead of the first real qkv groups
        warm_ps = ps_pv.tile([128, 512], f32, name="warm", tag="pv")
        for i in range(20):
            nc.tensor.matmul(
                warm_ps[0:64, 0:64], ones_f32[:, :], ones_f32[:, :],
                start=(i == 0), stop=(i == 19),
            )

        # ---- emission -----------------------------------------------------
        # Static per-engine instruction streams; qkv matmuls for the NEXT
        # window and projection matmuls for the PREVIOUS window are
        # interleaved between score/PV chunks as fill work that keeps the PE
        # array hot while ScalarE drains the exp chains.
        from collections import deque

        fill_q = deque()

        def fill(n):
            for _ in range(n):
                if fill_q:
                    fill_q.popleft()()

        def qk_group(w, xw, m):
            ps = ps_f.tile([128, 512], f32, name="psf", tag="psf")
            for k in range(FT):
                nc.tensor.matmul(
                    ps,
                    wq_sb[:, k, m * 128:(m + 1) * 128],
                    xw[:, k, :],
                    start=(k == 0), stop=(k == FT - 1),
                )
            if has_bqk:
                nc.scalar.activation(
                    out=qkT[:, w % 2, m, :], in_=ps, func=AF.Identity,
                    bias=bqk_sb[:, m:m + 1], scale=1.0,
                )
            else:
                nc.vector.tensor_copy(out=qkT[:, w % 2, m, :], in_=ps)

        def v_group(w, xw, mt, n):
            ps = ps_f.tile([128, 512], f32, name="psf", tag="psf")
            for k in range(FT):
                nc.tensor.matmul(
                    ps[:, 0:384],
                    xw[:, k, mt * 128:(mt + 1) * 128],
                    wq_sb[:, k, 2 * DIM + n * 384: 2 * DIM + (n + 1) * 384],
                    start=(k == 0), stop=(k == FT - 1),
                )
            nc.vector.tensor_copy(
                out=vsb[:, w % 2, mt, n * 6:(n + 1) * 6, :],
                in_=ps[:, 0:384].rearrange("p (j c) -> p j c", c=HD),
            )

        def push_qkv(w, xw=None):
            if xw is None:
                xw = xpool.tile([128, FT, NTOK], bf16, name="xw", tag="xw")
                nc.sync.dma_start(
                    out=xw[:, 0:3, :],
                    in_=xT_d[w, 0:384, :].rearrange("(k p) t -> p k t", p=128),
                )
                nc.sync.dma_start(
                    out=xw[:, 3:6, :],
                    in_=xT_d[w, 384:768, :].rearrange("(k p) t -> p k t", p=128),
                )
            for m in range(2 * FT):
                fill_q.append(lambda w=w, xw=xw, m=m: qk_group(w, xw, m))
            for n in range(2):
                for mt in range(KT):
                    fill_q.append(lambda w=w, xw=xw, mt=mt, n=n: v_group(w, xw, mt, n))

        def proj_group(w, mt):
            osb = opool.tile([128, DIM], f32, name="osb", tag="osb")
            for n in range(2):
                ps = ps_f.tile([128, 512], f32, name="psf", tag="psf")
                for k in range(FT):
                    nc.tensor.matmul(
                        ps[:, 0:384],
                        aoT[:, w % 2, k, mt * 128:(mt + 1) * 128],
                        wp_sb[:, k, n * 384:(n + 1) * 384],
                        start=(k == 0), stop=(k == FT - 1),
                    )
                nc.vector.tensor_copy(out=osb[:, n * 384:(n + 1) * 384], in_=ps[:, 0:384])
                nc.sync.dma_start(
                    out=out_d[w, mt * 128:(mt + 1) * 128, n * 384:(n + 1) * 384],
                    in_=osb[:, n * 384:(n + 1) * 384],
                )

        def push_proj(w, mts=range(KT)):
            for mt in mts:
                fill_q.append(lambda w=w, mt=mt: proj_group(w, mt))

        # pending attention state: list of dicts for head-pairs whose em is
        # computed but PV/normalize has not been emitted yet
        live = []

        def emit_pv(st):
            w2 = st["w"] % 2
            t = st["hp"]
            pv = ps_pv.tile([128, 512], f32, name="pv", tag="pv")
            for kt in range(KT):
                nc.tensor.matmul(
                    pv[0:64, :],
                    vsb[:, w2, kt, 2 * t, :],
                    st["em_e"][:, kt, :],
                    start=(kt == 0), stop=(kt == KT - 1),
                    tile_position=(0, 0),
                )
                nc.tensor.matmul(
                    pv[64:128, :],
                    vsb[:, w2, kt, 2 * t + 1, :],
                    st["em_o"][:, kt, :],
                    start=(kt == 0), stop=(kt == KT - 1),
                    tile_position=(0, 64),
                )
            un = uopool.tile([128, NTOK], bf16, name="un", tag="un")
            nc.scalar.copy(out=un, in_=pv)
            st["un"] = un

        def emit_sums_norm(g0, g1, fast=False):
            # softmax denominators for 4 heads at once: 4-way col-tiled
            # ones-matmul (128x32 mode), one PSUM bank, partitions 0/32/64/96
            ems = [g0["em_e"], g0["em_o"], g1["em_e"], g1["em_o"]]
            sp = ps_sum.tile([128, 512], f32, name="sums", tag="pss")
            for kt in range(KT):
                for j, em in enumerate(ems):
                    nc.tensor.matmul(
                        sp[32 * j:32 * j + 1, :],
                        ones_sb[:, 0:1],
                        em[:, kt, :],
                        start=(kt == 0), stop=(kt == KT - 1),
                        tile_position=(0, 32 * j),
                    )
            ssb = sbcpool.tile([128, NTOK], f32, name="ssb", tag="ssb")
            nc.scalar.copy(out=ssb, in_=sp)   # evacuate sums, free the bank
            rc = rcpool.tile([128, NTOK], f32, name="rc", tag="rc")
            # reciprocal_approx_fast misreads PSUM sources — feed it from SBUF
            nc.vector.reciprocal_approx_fast(out=rc, in_=ssb)
            if fast:
                # tail path: PE is idle after the last scores, and the DMA
                # round-trip's per-hop semaphore latency (~3x0.7us) would be
                # the critical path — broadcast 1/S with K=1 outer-product
                # matmuls instead
                for jb, st in ((0, g0), (2, g1)):
                    rb = ps_s.tile([128, 512], f32, name="rbps", tag="pss")
                    for h in range(2):
                        j = jb + h
                        nc.tensor.matmul(
                            rb[64 * h:64 * h + 64, :],
                            ones_f32[32 * j:32 * j + 1, :],
                            rc[32 * j:32 * j + 1, :],
                            start=True, stop=True,
                            tile_position=(32 * j, 64 * h),
                        )
                    nc.vector.tensor_mul(
                        out=aoT[:, st["w"] % 2, st["hp"], :], in0=st["un"], in1=rb,
                    )
                return
            rd = drampool.tile([4, 512], f32, name="rdram", tag="rdram")
            for j in range(4):
                nc.sync.dma_start(out=rd[j:j + 1, :], in_=rc[32 * j:32 * j + 1, :])
            for jb, st in ((0, g0), (2, g1)):
                # partition-broadcast 1/S across the pair's 128 partitions via
                # a DRAM round-trip: DMA reads from DRAM may use partition
                # stride 0, and the DMA engines are otherwise idle
                rb = rbpool.tile([128, NTOK], f32, name="rb", tag="rb")
                nc.sync.dma_start(out=rb[0:64, :], in_=rd[jb:jb + 1, :].partition_broadcast(64))
                nc.sync.dma_start(out=rb[64:128, :], in_=rd[jb + 1:jb + 2, :].partition_broadcast(64))
                nc.vector.tensor_mul(
                    out=aoT[:, st["w"] % 2, st["hp"], :], in0=st["un"], in1=rb,
                )

        def flush_pending(fast=False):
            # emit PV for the newest pending pair; when it completes an
            # even/odd head-pair group, emit sums+normalize for the group
            if not live:
                return False
            st = live[-1]
            if "un" not in st:
                emit_pv(st)
            if st["hp"] % 2 == 1 and len(live) >= 2:
                emit_sums_norm(live[-2], live[-1], fast=fast)
                del live[:]
                return True
            return False

        def emit_attn(w, hp):
            w2 = w % 2
            em_e = empool.tile([128, KT, NTOK], bf16, name="em", tag="em")
            em_o = empool.tile([128, KT, NTOK], bf16, name="em", tag="em")

            def score(kt):
                ps_e = ps_s.tile([128, 512], f32, name="pss", tag="pss")
                ps_o = ps_s.tile([128, 512], f32, name="pss", tag="pss")
                nc.tensor.matmul(
                    ps_e,
                    qkT[0:64, w2, FT + hp, kt * 128:(kt + 1) * 128],
                    qkT[0:64, w2, hp, :],
                    start=True, stop=True, tile_position=(0, 0),
                )
                nc.tensor.matmul(
                    ps_o,
                    qkT[64:128, w2, FT + hp, kt * 128:(kt + 1) * 128],
                    qkT[64:128, w2, hp, :],
                    start=True, stop=True, tile_position=(64, 0),
                )
                nc.scalar.activation(out=em_e[:, kt, :], in_=ps_e, func=AF.Exp)
                nc.scalar.activation(out=em_o[:, kt, :], in_=ps_o, func=AF.Exp)

            score(0)
            score(1)
            fill(2)
            score(2)
            score(3)
            nc.vector.tensor_mul(out=em_e, in0=em_e, in1=eb_sb[:, 2 * hp, :, :])
            nc.vector.tensor_mul(out=em_o, in0=em_o, in1=eb_sb[:, 2 * hp + 1, :, :])
            did_sums = flush_pending()
            if w == 2:
                # under-drain window 2 so window 3 (which has no next-window
                # qkv to interleave) inherits fill work instead of starving
                fill(1)
            else:
                fill(3 if did_sums else 2)
            live.append({"w": w, "hp": hp, "em_e": em_e, "em_o": em_o})

        # prologue: first window's qk groups and the first half of its v
        # groups are emitted inline; the remaining v groups stay in the fill
        # queue so window 0's attention has enough fill work (there is no
        # previous window's projection to interleave yet)
        push_qkv(0, xw=xw0)
        fill(16)
        for w in range(BW):
            if w + 1 < BW:
                push_qkv(w + 1)
            for hp in range(NHP):
                emit_attn(w, hp)
                if hp == 0 and w > 0:
                    # window w-1's attn-out finished during emit_attn above;
                    # stagger the final windows' proj so the tail of the
                    # last attention chain keeps some fill work
                    if w == BW - 1:
                        push_proj(w - 1, range(2))
                    else:
                        push_proj(w - 1)
                if w == BW - 1 and hp == 2:
                    push_proj(w - 1, range(2, 3))
        flush_pending(fast=True)
        # the last proj(BW-2) group and any leftovers execute on the PE while
        # the final sums/reciprocal/normalize chain drains on ScalarE/DVE
        push_proj(BW - 2, range(3, KT))
        fill(len(fill_q))
        push_proj(BW - 1)
        fill(len(fill_q))

    if not nc.is_finalized():
        nc.finalize()
    return nc


def _host_prep(x, Wqkv, bqkv, rel_pos_bias_table, rel_pos_index):
    bf16 = ml_dtypes.bfloat16
    x = np.asarray(x, np.float32)
    Wqkv = np.asarray(Wqkv, np.float32)
    bqkv = np.asarray(bqkv, np.float32)
    table = np.asarray(rel_pos_bias_table, np.float32)
    idx = np.asarray(rel_pos_index)

    wqkvT = Wqkv.T.copy()               # [768, 2304]
    wqkvT[:, :DIM] *= SCALE             # fold softmax scale into q weights
    wqkvT_bf = wqkvT.astype(bf16)

    bqk = bqkv[:2 * DIM].copy()
    bqk[:DIM] *= SCALE
    has_bqk = bool(np.any(bqk))
    bqk_packed = np.ascontiguousarray(bqk.reshape(2 * FT, 128).T, dtype=np.float32)

    # expb[p, h, kt, q] = exp(bias_h[q, k]) with k = kt*128+p  (scores are transposed)
    E = np.exp(table[idx])              # [q, k, h]
    eb = E.transpose(1, 2, 0)           # [k, h, q]
    eb = eb.reshape(KT, 128, NH, NTOK).transpose(1, 2, 0, 3)   # [p, h, kt, q]
    eb_bf = np.ascontiguousarray(eb, dtype=bf16)

    xT = x.reshape(NCORES, BW, NTOK, DIM).transpose(0, 1, 3, 2)  # [core, w, feat, tok]
    xT_bf = np.ascontiguousarray(xT, dtype=bf16)
    return xT_bf, wqkvT_bf, bqk_packed, has_bqk, eb_bf


def kernel(x, Wqkv, bqkv, rel_pos_bias_table, rel_pos_index, Wproj, bproj):
    global LAST_RESULT
    from concourse.bass_utils import run_bass_kernel_spmd

    Wproj = np.asarray(Wproj, np.float32)
    bproj = np.asarray(bproj, np.float32)
    bqkv_np = np.asarray(bqkv, np.float32)

    xT_bf, wqkvT_bf, bqk_packed, has_bqk, eb_bf = _host_prep(
        x, Wqkv, bqkv_np, rel_pos_bias_table, rel_pos_index
    )
    wprojT_bf = np.ascontiguousarray(Wproj.T, dtype=ml_dtypes.bfloat16)

    key = has_bqk
    if key not in _nc_cache:
        _nc_cache[key] = _build(has_bqk)
    nc = _nc_cache[key]

    in_maps = [
        {
            "xT": xT_bf[c],
            "wqkvT": wqkvT_bf,
            "wprojT": wprojT_bf,
            "expb": eb_bf,
            "bqk": bqk_packed,
        }
        for c in range(NCORES)
    ]
    res = run_bass_kernel_spmd(
        nc, in_maps, list(range(NCORES)),
        trace=TRACE, trace_cores=[0] if TRACE else None,
    )
    LAST_RESULT = res
    out = np.concatenate([res.results[c]["out"] for c in range(NCORES)], axis=0)

    # v-bias and proj-bias enter the output linearly; apply exactly on host.
    corr = bproj + bqkv_np[2 * DIM:] @ Wproj.T
    if np.any(corr):
        out = out + corr.astype(np.float32)
    return np.ascontiguousarray(out, dtype=np.float32)
